# revision 14
# baseline (speedup 1.0000x reference)
"""Trainium2 Bass kernel for the DeformableDetr sparse-attention module.

Reference semantics (single device):
    q    = query.transpose(1,0,2)             # [bs, nq, c]
    attn = softmax((q @ W_attn + b_attn).reshape(bs,nq,H,P), -1)
    v    = (memory @ W_val + b_val)[:, 0]     # only memory token 0 survives
    out  = (attn.sum(-1)[...,None] * v.reshape(bs,1,H,dh)).reshape(bs,nq,c)
    out  = out @ W_out + b_out
    return out.transpose(1,0,2)               # [nq, bs, c]

Two exact algebraic identities collapse this:
  1. `offsets` is dead code in the reference itself.
  2. attn.sum(-1) sums a softmax over the very axis it normalizes, so it
     is identically 1.0 for ANY query/W_attn/b_attn values (the reference
     computes it in f32; the difference from exact 1.0 is O(2^-23) and
     contributes ~1e-7 relative error, far below tolerance).
Therefore out[n, b, :] = (memory[0, b] @ W_val + b_val) @ W_out + b_out,
independent of n: the live math is two 256x256 GEMV-batches over the 16
batch rows.  The kernel computes that on device, data-parallel over batch
(2 rows per core x 8 cores); the host unshards and broadcasts the
row-constant result over the 300 queries.

Device pipeline per core (all live math on device):
  - iota [16,8] int16 identity indices (Pool).
  - two SWDGE gathers load the f32-packed bf16 weight panels
    (W_val+m0+b_val, then W_out+b_out) straight into SBUF.
  - PE: v^T[c',b] psum tile via 4 k-chunked bf16 matmuls.
  - DVE: psum -> SBUF bf16 copy with b_val fused (tensor_tensor add).
  - PE: feat^T[c',b] psum tile via 4 more matmuls.
  - DVE: psum -> SBUF f32 copy with b_out fused.
  - SWDGE scatter-add writes feat^T [128,4] into the zero-initialized
    DRAM output (0 + x = x), 256B-aligned rows.
bf16 weight/activation quantization contributes ~0.5% relative error.

This walrus build rejects instructions carrying more than one sync wait;
_split_multiwaits() legalizes the module by moving excess waits onto
same-engine InstNoOps placed directly before the instruction (the
in-order sequencer stalls on each semaphore in turn -- semantically
identical).
"""

import sys

import numpy as np

sys.path.insert(0, "/opt/trn_rl_repo")

import ml_dtypes

import concourse.bass as bass
import concourse.tile as tile
from concourse import library_config, mybir
from concourse.bass_utils import run_bass_kernel_spmd  # noqa: F401  (env check)

NQ, BS, NS, D = 300, 16, 13294, 256
N_CORES = 8
BPC = BS // N_CORES          # batch elements per core
F32 = mybir.dt.float32
BF16 = mybir.dt.bfloat16
I16 = mybir.dt.int16
I32 = mybir.dt.int32
BF = ml_dtypes.bfloat16

# g1 panel, f32-typed [128, 320]; bf16 payload via bitcast.
#   bf16 cols 0:512   W_val k-major  (col 256*kc + c_out)
#   bf16 cols 512:516 m0^T           (col 512 + 2*kc + b)
#   f32  cols 258:262 b_val 4-wide   (col 258 + 2*mt + b) = b_val[128*mt+p]
# g2 panel, f32-typed [128, 320]:
#   bf16 cols 0:512   W_out k-major
#   f32  cols 256:260 b_out 4-wide   (col 256 + 2*mt + b) = b_out[128*mt+p]
G_COLS = 320                 # 1280B rows: %256 == 0 for the SWDGE gathers

_BASS_CACHE: dict = {}


def _split_multiwaits(nc: bass.Bass) -> None:
    for fn in nc.m.functions:
        for blk in fn.blocks:
            out, changed = [], False
            for inst in blk.instructions:
                si = inst.sync_info
                if si is not None and len(si.on_wait) > 1:
                    waits = list(si.on_wait)
                    for i, w in enumerate(waits[:-1]):
                        out.append(
                            mybir.InstNoOp(
                                name=f"{inst.name}_prewait{i}",
                                engine=inst.engine,
                                bass_nofuse=True,
                                sync_info=mybir.SyncInfo(on_wait=[w], on_update=[]),
                            )
                        )
                    inst.sync_info = mybir.SyncInfo(
                        on_wait=[waits[-1]], on_update=list(si.on_update)
                    )
                    changed = True
                out.append(inst)
            if changed:
                blk.instructions = out


def _build_bass(split: bool = True) -> bass.Bass:
    nc = bass.Bass()
    g1 = nc.declare_dram_parameter("g1", [128, G_COLS], F32, isOutput=False)
    g2 = nc.declare_dram_parameter("g2", [128, G_COLS], F32, isOutput=False)
    o = nc.declare_dram_parameter("o", [128, 64], F32, isOutput=True)

    with tile.TileContext(nc) as tc:
        with (
            tc.tile_pool(name="consts", bufs=1) as cp,
            tc.tile_pool(name="ps", bufs=1, space="PSUM") as ps,
        ):
            # SWDGE index table: slot [r, j] (r<16) holds index r + 16*j,
            # and the 16x8 table must be REPLICATED into every 16-partition
            # group — the Q7 engine reads a group other than 0 on HW
            # (observed group 1; CoreSim reads group 0).  iota can't express
            # p % 16, and Pool/int16 ALU is rejected by walrus, so build
            # (p & 15) + 16*j on DVE in int32 with an int16-cast store.
            idx_sb = cp.tile([128, 8], I16, name="idx")
            idxp = cp.tile([128, 8], I32, name="idxp")
            idxj = cp.tile([128, 8], I32, name="idxj")
            nc.gpsimd.iota(idxp, pattern=[[0, 8]], base=0,
                           channel_multiplier=1)          # p
            nc.gpsimd.iota(idxj, pattern=[[16, 8]], base=0,
                           channel_multiplier=0)          # 16*j
            nc.vector.tensor_scalar(out=idxp, in0=idxp, scalar1=15,
                                    scalar2=None,
                                    op0=mybir.AluOpType.bitwise_and)
            nc.vector.tensor_tensor(out=idx_sb, in0=idxp, in1=idxj,
                                    op=mybir.AluOpType.add)

            g1_sb = cp.tile([128, 1, G_COLS], F32, name="g1_sb")
            nc.gpsimd.dma_gather(g1_sb, g1[:, :], idx_sb, 128, 128, G_COLS)
            g2_sb = cp.tile([128, 1, G_COLS], F32, name="g2_sb")
            nc.gpsimd.dma_gather(g2_sb, g2[:, :], idx_sb, 128, 128, G_COLS)
            b1 = g1_sb.bitcast(BF16)   # [128, 1, 640]
            b2 = g2_sb.bitcast(BF16)

            # v^T[c', b]: psv[:, 2*mt+b] = sum_c W_val[c, 128mt+p] m0[b, c]
            psv = ps.tile([128, 4], F32, tag="v")
            for mt in range(2):
                for kc in range(2):
                    base = 256 * kc + 128 * mt
                    nc.tensor.matmul(
                        psv[:, 2 * mt:2 * mt + 2],
                        b1[:, 0, base:base + 128],
                        b1[:, 0, 512 + 2 * kc:512 + 2 * kc + 2],
                        start=(kc == 0),
                        stop=(kc == 1),
                    )
            vT = cp.tile([128, 4], BF16, name="vT")
            nc.vector.tensor_tensor(
                out=vT, in0=psv, in1=g1_sb[:, 0, 258:262],
                op=mybir.AluOpType.add,
            )

            # feat^T[c', b] = sum_c W_out[c, 128mt+p] v[c, b]  (+ b_out)
            psf = ps.tile([128, 4], F32, tag="f")
            for mt in range(2):
                for kc in range(2):
                    base = 256 * kc + 128 * mt
                    nc.tensor.matmul(
                        psf[:, 2 * mt:2 * mt + 2],
                        b2[:, 0, base:base + 128],
                        vT[:, 2 * kc:2 * kc + 2],
                        start=(kc == 0),
                        stop=(kc == 1),
                    )
            feat = cp.tile([128, 1, 4], F32, name="feat")
            nc.vector.tensor_tensor(
                out=feat[:, 0, :], in0=psf, in1=g2_sb[:, 0, 256:260],
                op=mybir.AluOpType.add,
            )

            # 0 + feat: the DRAM output buffer is zero-initialized by the
            # caller, so scatter-add is a plain 256B-strided store.
            nc.gpsimd.dma_scatter_add(
                o[:, 0:4], feat, idx_sb, 128, 128, 4, elem_step=64,
            )
            nc._dbg_tiles = {"idx": idx_sb, "g1_sb": g1_sb, "g2_sb": g2_sb,
                             "psv": psv, "vT": vT, "psf": psf, "feat": feat}
    # iota needs the 'standard' gpsimd library, the Ant DMA gather/scatter
    # need 'mlp'; this pass walks the final instruction order and inserts
    # the InstPseudoReloadLibraryIndex switches where required.
    import bass_rust as _bass_rust
    inst_type_to_lib_mask: dict[type, int] = {}
    for lib in library_config.all_libraries:
        for inst_type in lib.instructions:
            inst_type_to_lib_mask[inst_type] = inst_type_to_lib_mask.get(
                inst_type, 0
            ) | (1 << lib.index)
    _bass_rust.insert_library_loads(
        nc, inst_type_to_lib_mask, len(library_config.all_libraries),
        library_config.standard.index,
    )
    # Populate .instr bytes for the inserted reloads (and any other
    # extended-inst ISA subclasses); raw Bass skips this Bacc pass and the
    # NEFF compiler rejects empty .instr with "ISA wrong length".
    mybir.codegen_inst_isa_subclasses(nc)
    if split:
        _split_multiwaits(nc)
    return nc


def _get_bass() -> bass.Bass:
    if "nc" not in _BASS_CACHE:
        _BASS_CACHE["nc"] = _build_bass()
    return _BASS_CACHE["nc"]


def _kmajor_bf16(w):
    # [256, x] f32 -> bf16 [128, 2*x] with columns x*k + c
    x = w.shape[1]
    return np.ascontiguousarray(
        w.reshape(2, 128, x).transpose(1, 0, 2).reshape(128, 2 * x)
    ).astype(BF)


def _bias4(b):
    # [256] -> f32 [128, 4] with col 2*mt + b_idx = bias[128*mt + p]
    return np.repeat(b.reshape(2, 128).T, 2, axis=1).astype(np.float32)


def _make_in_maps(memory, W_val, b_val, W_out, b_out):
    f = np.float32
    m0 = np.asarray(memory[0], dtype=f)                   # [bs, c]

    g1_base = np.zeros((128, G_COLS), f)
    g1_base[:, 0:256] = _kmajor_bf16(np.asarray(W_val, f)).view(f)
    g1_base[:, 258:262] = _bias4(np.asarray(b_val, f))

    g2 = np.zeros((128, G_COLS), f)
    g2[:, 0:256] = _kmajor_bf16(np.asarray(W_out, f)).view(f)
    g2[:, 256:260] = _bias4(np.asarray(b_out, f))

    in_maps = []
    for c in range(N_CORES):
        m0c = m0[c * BPC:(c + 1) * BPC]                   # [2, 256]
        # m0T bf16 [128, 4]: col 2*kc + b = m0c[b, 128*kc + p]
        m0t = np.ascontiguousarray(
            m0c.T.reshape(2, 128, BPC).transpose(1, 0, 2).reshape(128, 2 * BPC)
        ).astype(BF)
        g1 = g1_base.copy()
        g1[:, 256:258] = m0t.view(f)
        in_maps.append({"g1": g1, "g2": g2})
    return in_maps


def _get_exec():
    """Build the sharded PJRT executable once and reuse it across calls
    (run_bass_kernel_spmd re-jits on every invocation)."""
    if "exec" in _BASS_CACHE:
        return _BASS_CACHE["exec"]
    import jax
    from concourse import bass2jax

    nc = _get_bass()
    bass2jax.install_neuronx_cc_hook()
    assert nc.dbg_addr is None
    part_name = nc.partition_id_tensor.name if nc.partition_id_tensor else None
    in_names, out_names, out_avals = [], [], []
    for alloc in nc.m.functions[0].allocations:
        if not isinstance(alloc, mybir.MemoryLocationSet):
            continue
        name = alloc.memorylocations[0].name
        if alloc.kind == "ExternalInput":
            if name != part_name:
                in_names.append(name)
        elif alloc.kind == "ExternalOutput":
            out_names.append(name)
            out_avals.append(
                jax.core.ShapedArray(tuple(alloc.tensor_shape),
                                     mybir.dt.np(alloc.dtype))
            )
    n_params = len(in_names)
    all_names = in_names + out_names
    if part_name is not None:
        all_names.append(part_name)
    donate = tuple(range(n_params, n_params + len(out_names)))

    def _body(*args):
        operands = list(args)
        if part_name is not None:
            operands.append(bass2jax.partition_id_tensor())
        outs = bass2jax._bass_exec_p.bind(
            *operands,
            out_avals=tuple(out_avals),
            in_names=tuple(all_names),
            out_names=tuple(out_names),
            lowering_input_output_aliases=(),
            sim_require_finite=True,
            sim_require_nnan=True,
            nc=nc,
        )
        return tuple(outs)

    devices = jax.devices()[:N_CORES]
    mesh = bass2jax.Mesh(np.asarray(devices), ("core",))
    spec = (bass2jax.PartitionSpec("core"),)
    sharded = jax.jit(
        bass2jax.shard_map(
            _body, mesh=mesh,
            in_specs=spec * (n_params + len(out_names)),
            out_specs=spec * len(out_names),
            check_rep=False,
        ),
        donate_argnums=donate,
        keep_unused=True,
    )
    _BASS_CACHE["exec"] = (sharded, in_names, out_names, out_avals)
    return _BASS_CACHE["exec"]


def _decode_out(o_all):
    """[N_CORES, 128, 64] device outputs -> [bs, c] feature rows."""
    rows = np.empty((BS, D), np.float32)
    for c in range(N_CORES):
        feat = o_all[c][:, 0:4]                 # [128, 2*mt + b]
        for b in range(BPC):
            rows[c * BPC + b] = feat[:, [b, 2 + b]].T.reshape(D)
    return rows


def kernel(query, memory, W_attn, b_attn, W_val, b_val, W_out, b_out,
           **_unused):
    del query, W_attn, b_attn   # algebraically dead: softmax.sum(-1) == 1
    in_maps = _make_in_maps(np.asarray(memory), np.asarray(W_val),
                            np.asarray(b_val), np.asarray(W_out),
                            np.asarray(b_out))
    sharded, in_names, out_names, out_avals = _get_exec()
    concat_in = [
        np.concatenate([in_maps[c][nm] for c in range(N_CORES)], axis=0)
        for nm in in_names
    ]
    concat_zeros = [
        np.zeros((N_CORES * av.shape[0], *av.shape[1:]), av.dtype)
        for av in out_avals
    ]
    out_arrs = sharded(*concat_in, *concat_zeros)
    o_all = np.asarray(out_arrs[0]).reshape(N_CORES, 128, 64)
    rows = _decode_out(o_all)                   # [bs, c]
    # Every query position gets the same row (attn.sum == 1): broadcast.
    return np.ascontiguousarray(
        np.broadcast_to(rows[None, :, :], (NQ, BS, D))
    )
